# revision 28
# baseline (speedup 1.0000x reference)
"""ROI-Align + MLP classification head (nms_detection) on 8 Trainium2 cores.

Strategy: data-parallel over batch (2 images per core). Host pre-casts the
feature map to fp16 and stores it row-paired (fmP[b, y, x] = fm[b, y, x] ++
fm[b, y+1, x], 512 ch), so ONE 2KB gather descriptor fetches all 4 bilinear
corners of a sample. MLP weights are pre-cast/arranged fp16 on host; the
proposals ship as an fp16 hi/lo split laid out per roi-slot block so a
single K=16 selection matmul (SEL.T @ prop_rows) materializes exact fp32
sample-major coords on all 128 partitions. The index chain runs on the
vector engine ([128, 6] tiles), six indirect DMAs (128 descriptors each, no
gpsimd library needed) fetch the sample blocks, and the bilinear combine
(fp16, weights pre-expanded over channels for full DVE rate) + PE transpose
+ 3-layer MLP (fp16 in / fp32 psum) + fp32 softmax finish, pipelined per
2-group chunk.

Layouts (per core): 44 rois x 16 bin-centers = 704 samples.
  roi slot (h, g): roi = h*6 + g, h in 0..7, g in 0..5 (48 slots, 4 garbage)
  sample partition p = h*16 + q (q = iy*4+ix); gather block j = g (6 blocks)
  idx value = fmP row = b*(H-1)*W + y0*W + x0 (int32); each gather reads
  rows idx..idx+1 = pixels (x0, x0+1) x (row pair y0, y0+1) x 256 ch.
"""

import numpy as np

import concourse.bacc as bacc
import concourse.bass as bass
import concourse.mybir as mybir
import concourse.tile as tile
from concourse._compat import get_trn_type
from concourse.bass_utils import run_bass_kernel_spmd

# Problem shape (hardcoded per contract)
B, P, H, W, C = 16, 22, 128, 128, 256
NUM_CLASSES = 10
N_CORES = 8
B_LOC = B // N_CORES        # 2 images per core
NROI = B_LOC * P            # 44 rois per core
NRS = 48                    # roi slots (8 partition-blocks x 6 groups)
NG = 6                      # roi-slot groups
HID1, HID2 = 128, 64
F32 = mybir.dt.float32
F16 = mybir.dt.float16
I32 = mybir.dt.int32
AX_X = mybir.AxisListType.X
OP = mybir.AluOpType
AF = mybir.ActivationFunctionType

HP = H - 1                      # 127 paired rows per image
NPROW = B_LOC * HP * W          # 32512 fmP pixel rows per core
MAX_PIX = NPROW - 2             # last valid fmP row start
MAGIC = 12582912.0              # 1.5 * 2^23 fp32 round-to-int magic
CHUNKS = [(0, 3), (3, 5), (5, 6)]   # combine/matmul chunk group-ranges
NCH = len(CHUNKS)


def _static_consts():
    ident = np.eye(128).astype(np.float16)
    # SEL[k, p] = 1 iff p//16 == k//2: rows 2h (hi) and 2h+1 (lo) of the
    # host-prepared prop_sm both route to partition block h.
    sel = np.zeros((16, 128), np.float16)
    for k in range(16):
        sel[k, (k // 2) * 16:(k // 2 + 1) * 16] = 1.0
    p = np.arange(128)
    q = p % 16
    cy = ((q // 4).astype(np.float32) + 0.5) / 4.0
    cx = ((q % 4).astype(np.float32) + 0.5) / 4.0
    h = (p // 16)[:, None]
    g = np.arange(NG)[None, :]
    roi = h * 6 + g
    bofs = np.where(roi >= P, float(HP * W), 0.0).astype(np.float32)  # [128,6]
    cb = np.concatenate([cy[:, None], cx[:, None], bofs], axis=1)
    return ident, sel, np.ascontiguousarray(cb.astype(np.float32))


def emit_kernel(nc, tc, fm, prop, W1, b1, W2, b2, W3, b3, out, consts):
    """Emit the per-core tile kernel. All args are bass.APs."""
    with (
        tc.tile_pool(name="const", bufs=1) as cpool,
        tc.tile_pool(name="work", bufs=1) as wpool,
        tc.tile_pool(name="psum", bufs=1, space="PSUM") as ppool,
    ):
        _emit_body(nc, tc, fm, prop, W1, b1, W2, b2, W3, b3, out, consts,
                   cpool, wpool, ppool)


def _emit_body(nc, tc, fm, prop, W1, b1, W2, b2, W3, b3, out, consts,
               cpool, wpool, ppool):
    ident_c, sel_c, cb_c = consts
    V = nc.vector

    # ---------------- coords via selection matmul ----------------
    # psCB[p, (g, k)] = sum_k SEL[k, p] * prop_sm[k, (g, k4)] reconstructs
    # exact fp32 coords for partition block h = p//16 (hi+lo fp16 rows).
    phl = wpool.tile([16, 24], F16, name="phl")
    nc.sync.dma_start(phl[:], prop)
    sel = cpool.tile([16, 128], F16, name="sel")
    nc.sync.dma_start(sel[:], sel_c)
    cb = cpool.tile([128, 8], F32, name="cb")
    nc.sync.dma_start(cb[:], cb_c)
    ident = cpool.tile([128, 128], F16, name="ident")
    psCB = ppool.tile([128, 24], F32, name="psCB")
    nc.tensor.matmul(out=psCB[:], lhsT=sel[:], rhs=phl[:], start=True,
                     stop=True)
    CB = cpool.tile([128, 24], F32, name="CB")
    V.tensor_copy(out=CB[:], in_=psCB[:])

    # scalar queue: remaining loads; W1 (the big one) last.
    nc.scalar.dma_start(ident[:], ident_c)
    W2sb = cpool.tile([128, HID2], F16, name="W2sb")
    nc.scalar.dma_start(W2sb[:], W2)
    W3sb = cpool.tile([HID2, NUM_CLASSES], F16, name="W3sb")
    nc.scalar.dma_start(W3sb[:], W3)
    b1sb = cpool.tile([128, 1], F32, name="b1sb")
    nc.scalar.dma_start(b1sb[:], b1.rearrange("(p o) -> p o", o=1))
    b2sb = cpool.tile([HID2, 1], F32, name="b2sb")
    nc.scalar.dma_start(b2sb[:], b2.rearrange("(p o) -> p o", o=1))
    b3sb = cpool.tile([NROI, NUM_CLASSES], F32, name="b3sb")
    nc.scalar.dma_start(b3sb[:], b3.unsqueeze(0).to_broadcast([NROI, NUM_CLASSES]))
    W1sb = cpool.tile([128, 4096], F16, name="W1sb")
    nc.scalar.dma_start(W1sb[:], W1)

    # ---------------- index chain (critical path to the gathers) --------
    # Sample-major [128, (g, yx)] views.
    cgv = CB[:, :].rearrange("p (g k) -> p g k", g=NG)
    bofs = cb[:, 2:8]
    dyx = wpool.tile([128, 12], F32, name="dyx")
    syx = wpool.tile([128, 12], F32, name="syx")
    f0m = wpool.tile([128, 12], F32, name="f0m")
    f0 = wpool.tile([128, 12], F32, name="f0")
    pixb = wpool.tile([128, NG], F32, name="pixb")
    dv = dyx[:, :].rearrange("p (g yx) -> p g yx", yx=2)
    sv_ = syx[:, :].rearrange("p (g yx) -> p g yx", yx=2)
    f0v = f0[:, :].rearrange("p (g yx) -> p g yx", yx=2)
    cyx = cb[:, 0:2].unsqueeze(1).to_broadcast([128, NG, 2])

    V.tensor_tensor(out=dv[:], in0=cgv[:, :, 2:4], in1=cgv[:, :, 0:2],
                    op=OP.subtract)
    V.tensor_tensor(out=sv_[:], in0=dv[:], in1=cyx, op=OP.mult)
    V.tensor_tensor(out=sv_[:], in0=sv_[:], in1=cgv[:, :, 0:2], op=OP.add)
    # f0 = round(s - 0.5) via fp32 magic; consistent-pair bilinear stays exact
    V.tensor_scalar(out=f0m[:], in0=syx[:], scalar1=-0.5, scalar2=MAGIC,
                    op0=OP.add, op1=OP.add)
    V.tensor_scalar(out=f0[:], in0=f0m[:], scalar1=-MAGIC, scalar2=None,
                    op0=OP.add)
    # pixb = b*HP*W + y0*W + x0, clamped
    V.tensor_scalar(out=pixb[:], in0=f0v[:, :, 0], scalar1=float(W),
                    scalar2=None, op0=OP.mult)
    V.tensor_tensor(out=pixb[:], in0=pixb[:], in1=f0v[:, :, 1], op=OP.add)
    V.tensor_tensor(out=pixb[:], in0=pixb[:], in1=bofs, op=OP.add)
    V.tensor_scalar(out=pixb[:], in0=pixb[:], scalar1=0.0,
                    scalar2=float(MAX_PIX), op0=OP.max, op1=OP.min)
    idx = cpool.tile([128, NG], I32, name="gidx")
    V.tensor_copy(out=idx[:], in_=pixb[:])

    # ---------------- gathers: 6 indirect DMAs (128 descriptors) ---------
    # G[p, (g, x, ab, c)] fp16; each descriptor reads fmP rows idx, idx+1 =
    # pixels (x0, x0+1) x (row pair) x 256 ch. No gpsimd library needed.
    G = wpool.tile([128, NG * 1024], F16, name="gather")
    fmr = fm.rearrange("b h w c -> (b h w) c")            # [32512, 512]
    for j in range(NG):
        nc.gpsimd.indirect_dma_start(
            out=G[:, j * 1024:(j + 1) * 1024],
            out_offset=None,
            in_=fmr,
            in_offset=bass.IndirectOffsetOnAxis(ap=idx[:, j:j + 1], axis=0),
        )

    # ------- bilinear corner weights (off the gather critical path) -------
    lyx = wpool.tile([128, 12], F32, name="lyx")
    hyx = wpool.tile([128, 12], F32, name="hyx")
    V.tensor_tensor(out=lyx[:], in0=syx[:], in1=f0[:], op=OP.subtract)
    V.tensor_scalar(out=hyx[:], in0=lyx[:], scalar1=-1.0, scalar2=1.0,
                    op0=OP.mult, op1=OP.add)
    lv = lyx[:, :].rearrange("p (g yx) -> p g yx", yx=2)
    hv = hyx[:, :].rearrange("p (g yx) -> p g yx", yx=2)
    ly, lx = lv[:, :, 0], lv[:, :, 1]
    hy, hx = hv[:, :, 0], hv[:, :, 1]
    # wc[p, (g, x, ab)] fp16 (matches the fmP elem layout x-outer)
    wc = cpool.tile([128, 24], F16, name="wcat")
    wv = wc[:, :].rearrange("p (g x ab) -> p g x ab", x=2, ab=2)
    V.tensor_tensor(out=wv[:, :, 0, 0], in0=hy, in1=hx, op=OP.mult)
    V.tensor_tensor(out=wv[:, :, 0, 1], in0=ly, in1=hx, op=OP.mult)
    V.tensor_tensor(out=wv[:, :, 1, 0], in0=hy, in1=lx, op=OP.mult)
    V.tensor_tensor(out=wv[:, :, 1, 1], in0=ly, in1=lx, op=OP.mult)
    # wbig[cix]: wc chunk expanded over channels so the combine multiply
    # reads contiguous fp16 at full DVE rate (broadcast reads run ~2x slow).
    wbig = [wpool.tile([128, (g1 - g0) * 1024], F16, name=f"wbig{c}")
            for c, (g0, g1) in enumerate(CHUNKS)]
    for cix, (g0, g1) in enumerate(CHUNKS):
        ng = g1 - g0
        src = wc[:, g0 * 4:g1 * 4] \
            .rearrange("p (g x ab) -> p g x ab", x=2, ab=2).unsqueeze(4) \
            .to_broadcast([128, ng, 2, 2, C])
        dst = wbig[cix][:, :].rearrange("p (g x ab c) -> p g x ab c",
                                        g=ng, x=2, ab=2)
        if cix % 2 == 0:
            V.tensor_copy(out=dst, in_=src)
        else:
            nc.scalar.copy(out=dst, in_=src)

    # ---------------- bilinear combine + transpose, per 2-group chunk -----
    Gv = G[:, :].rearrange("p (g x ab c) -> p g x ab c", g=NG, x=2, ab=2)
    sv2 = wpool.tile([128, NG * 512], F16, name="sv2")
    sv = wpool.tile([128, NG * 256], F16, name="sv")
    s2v = sv2[:, :].rearrange("p (g x c) -> p g x c", g=NG, x=2)
    svv = sv[:, :].rearrange("p (g c) -> p g c", g=NG)
    svT = [wpool.tile([128, NG * 128], F16, name=f"svT{h}") for h in range(2)]
    # layer-1 psum per chunk: columns (a, b_chunk); l1 interleaves to (a, b)
    psum1 = [ppool.tile([128, 8 * (g1 - g0)], F32, name=f"psum1{c}")
             for c, (g0, g1) in enumerate(CHUNKS)]
    l1 = wpool.tile([128, NRS], F16, name="l1")
    l1v = l1[:, :].rearrange("p (a b) -> p a b", a=8)
    for cix, (g0, g1) in enumerate(CHUNKS):
        ng = g1 - g0
        gs = slice(g0, g1)
        V.tensor_tensor(out=Gv[:, gs], in0=Gv[:, gs],
                        in1=wbig[cix][:, :].rearrange(
                            "p (g x ab c) -> p g x ab c", g=ng, x=2, ab=2),
                        op=OP.mult)
        V.tensor_tensor(out=s2v[:, gs], in0=Gv[:, gs, :, 0], in1=Gv[:, gs, :, 1],
                        op=OP.add)
        V.tensor_tensor(out=svv[:, gs], in0=s2v[:, gs, 0], in1=s2v[:, gs, 1],
                        op=OP.add)
        for g in range(g0, g1):
            for h in range(2):
                pt = ppool.tile([128, 128], F16, tag="pt", bufs=3, name="pt")
                nc.tensor.transpose(
                    out=pt[:],
                    in_=sv[:, g * 256 + h * 128: g * 256 + (h + 1) * 128],
                    identity=ident[:])
                nc.scalar.copy(out=svT[h][:, g * 128:(g + 1) * 128],
                               in_=pt[:])
        # layer-1 matmul accumulation for this chunk's groups
        for h in range(2):
            for q in range(16):
                k = q * 2 + h
                rhs = svT[h][:, g0 * 128:g1 * 128] \
                    .rearrange("p (b a s) -> p a b s", b=ng, a=8)[:, :, :, q]
                nc.tensor.matmul(out=psum1[cix][:],
                                 lhsT=W1sb[:, k * 128:(k + 1) * 128],
                                 rhs=rhs, start=(h == 0 and q == 0),
                                 stop=(h == 1 and q == 15))
        # relu on the vector engine (scalar queue is congested here)
        V.tensor_scalar(out=l1v[:, :, g0:g1], in0=psum1[cix][:],
                        scalar1=b1sb[:, 0:1], scalar2=0.0,
                        op0=OP.add, op1=OP.max)

    # ---------------- MLP layers 2, 3 ----------------
    psum23 = ppool.tile([HID2, NRS + NUM_CLASSES], F32, name="psum23")
    psum2 = psum23[:, 0:NRS]
    psum3 = psum23[0:NRS, NRS:NRS + NUM_CLASSES]
    nc.tensor.matmul(out=psum2, lhsT=W2sb[:, :], rhs=l1[:], start=True,
                     stop=True)
    l2 = wpool.tile([HID2, NRS], F16, name="l2")
    V.tensor_scalar(out=l2[:], in0=psum2, scalar1=b2sb[:, 0:1], scalar2=0.0,
                    op0=OP.add, op1=OP.max)
    nc.tensor.matmul(out=psum3, lhsT=l2[:], rhs=W3sb[:], start=True,
                     stop=True)

    # ---------------- softmax (rows 0..43 only, fp32) ----------------
    # logits are O(10), so fp32 exp needs no max-subtraction.
    logits = wpool.tile([NROI, NUM_CLASSES], F32, name="logits")
    V.tensor_tensor(out=logits[:], in0=psum3[0:NROI, :], in1=b3sb[:], op=OP.add)
    ex = wpool.tile([NROI, NUM_CLASSES], F32, name="ex")
    nc.scalar.activation(out=ex[:], in_=logits[:], func=AF.Exp,
                         bias=0.0, scale=1.0)
    ssum = wpool.tile([NROI, 1], F32, name="ssum")
    V.tensor_reduce(out=ssum[:], in_=ex[:], axis=AX_X, op=OP.add)
    rinv = wpool.tile([NROI, 1], F32, name="rinv")
    V.reciprocal(rinv[:], ssum[:])
    probs = wpool.tile([NROI, NUM_CLASSES], F32, name="probs")
    V.tensor_scalar(out=probs[:], in0=ex[:], scalar1=rinv[:, 0:1],
                    scalar2=None, op0=OP.mult)

    nc.sync.dma_start(out.rearrange("b p c -> (b p) c"), probs[:])


def build_module():
    nc = bacc.Bacc(get_trn_type() or "TRN2", target_bir_lowering=False, debug=False)
    fm = nc.dram_tensor("feature_map", [B_LOC, HP, W, 2 * C], F16, kind="ExternalInput")
    prop = nc.dram_tensor("proposals", [16, 24], F16, kind="ExternalInput")
    W1 = nc.dram_tensor("W1", [128, 4096], F16, kind="ExternalInput")
    b1 = nc.dram_tensor("b1", [HID1], F32, kind="ExternalInput")
    W2 = nc.dram_tensor("W2", [HID1, HID2], F16, kind="ExternalInput")
    b2 = nc.dram_tensor("b2", [HID2], F32, kind="ExternalInput")
    W3 = nc.dram_tensor("W3", [HID2, NUM_CLASSES], F16, kind="ExternalInput")
    b3 = nc.dram_tensor("b3", [NUM_CLASSES], F32, kind="ExternalInput")
    out = nc.dram_tensor("out", [B_LOC, P, NUM_CLASSES], F32, kind="ExternalOutput")

    ident_np, sel_np, cb_np = _static_consts()
    ident_c = nc.inline_tensor(ident_np, name="c_ident")
    sel_c = nc.inline_tensor(sel_np, name="c_sel")
    cb_c = nc.inline_tensor(cb_np, name="c_cb")

    with tile.TileContext(nc) as tc:
        emit_kernel(nc, tc, fm[:], prop[:], W1[:], b1[:], W2[:], b2[:], W3[:],
                    b3[:], out[:], (ident_c[:], sel_c[:], cb_c[:]))
    nc.compile()
    return nc


_NC_CACHE = None


def _get_module():
    global _NC_CACHE
    if _NC_CACHE is None:
        _NC_CACHE = build_module()
    return _NC_CACHE


def _shard_inputs(inputs):
    fm16 = np.asarray(inputs["feature_map"], dtype=np.float32).astype(np.float16)
    # paired rows: fmP[b, y, x] = fm[b, y] ++ fm[b, y+1] per pixel
    fmP = np.concatenate([fm16[:, :-1], fm16[:, 1:]], axis=3)
    fmP = np.ascontiguousarray(fmP)
    props = np.asarray(inputs["proposals"], dtype=np.float32)
    # W1 rows k*128+p -> [p, k*128+j] fp16 so lhsT chunks are contiguous.
    W1h = np.ascontiguousarray(
        np.asarray(inputs["W1"], dtype=np.float32).reshape(32, 128, HID1)
        .transpose(1, 0, 2).reshape(128, 4096).astype(np.float16))
    W2h = np.ascontiguousarray(np.asarray(inputs["W2"], dtype=np.float32).astype(np.float16))
    W3h = np.ascontiguousarray(np.asarray(inputs["W3"], dtype=np.float32).astype(np.float16))
    b1h = np.ascontiguousarray(np.asarray(inputs["b1"], dtype=np.float32))
    b2h = np.ascontiguousarray(np.asarray(inputs["b2"], dtype=np.float32))
    b3h = np.ascontiguousarray(np.asarray(inputs["b3"], dtype=np.float32))
    in_maps = []
    for c in range(N_CORES):
        sl = slice(B_LOC * c, B_LOC * (c + 1))
        # prop_sm[2h]   = fp16 hi of coords for roi slots (h, g=0..5)
        # prop_sm[2h+1] = fp16 lo; h=7 slots hold rois [42, 43, 38..41].
        pf = props[sl].reshape(NROI, 4)
        rows = np.zeros((8, 6, 4), np.float32)
        for h in range(7):
            rows[h] = pf[h * 6:(h + 1) * 6]
        rows[7, 0:2] = pf[42:44]
        rows[7, 2:6] = pf[38:42]
        rows = rows.reshape(8, 24)
        hi = rows.astype(np.float16)
        lo = (rows - hi.astype(np.float32)).astype(np.float16)
        phl = np.zeros((16, 24), np.float16)
        phl[0::2] = hi
        phl[1::2] = lo
        in_maps.append({
            "feature_map": fmP[sl],
            "proposals": np.ascontiguousarray(phl),
            "W1": W1h, "b1": b1h,
            "W2": W2h, "b2": b2h,
            "W3": W3h, "b3": b3h,
        })
    return in_maps


def run(inputs, trace=False):
    """Run on all 8 cores; returns (output [16,22,10], BassKernelResults)."""
    nc = _get_module()
    res = run_bass_kernel_spmd(nc, _shard_inputs(inputs), core_ids=list(range(N_CORES)),
                               trace=trace)
    out = np.concatenate([r["out"] for r in res.results], axis=0)
    return out, res


def kernel(**inputs) -> np.ndarray:
    out, _ = run(inputs, trace=False)
    return out


# revision 29
# speedup vs baseline: 1.0154x; 1.0154x over previous
"""ROI-Align + MLP classification head (nms_detection) on 8 Trainium2 cores.

Strategy: data-parallel over batch (2 images per core). Host pre-casts the
feature map to fp16 and stores it row-paired (fmP[b, y, x] = fm[b, y, x] ++
fm[b, y+1, x], 512 ch), so ONE 2KB gather descriptor fetches all 4 bilinear
corners of a sample. MLP weights are pre-cast/arranged fp16 on host; the
proposals ship as an fp16 hi/lo split laid out per roi-slot block so a
single K=16 selection matmul (SEL.T @ prop_rows) materializes exact fp32
sample-major coords on all 128 partitions. The index chain runs on the
vector engine ([128, 6] tiles), six indirect DMAs (128 descriptors each, no
gpsimd library needed) fetch the sample blocks, and the bilinear combine
(fp16, weights pre-expanded over channels for full DVE rate) + PE transpose
+ 3-layer MLP (fp16 in / fp32 psum) + fp32 softmax finish, pipelined per
2-group chunk.

Layouts (per core): 44 rois x 16 bin-centers = 704 samples.
  roi slot (h, g): roi = h*6 + g, h in 0..7, g in 0..5 (48 slots, 4 garbage)
  sample partition p = h*16 + q (q = iy*4+ix); gather block j = g (6 blocks)
  idx value = fmP row = b*(H-1)*W + y0*W + x0 (int32); each gather reads
  rows idx..idx+1 = pixels (x0, x0+1) x (row pair y0, y0+1) x 256 ch.
"""

import numpy as np

import concourse.bacc as bacc
import concourse.bass as bass
import concourse.mybir as mybir
import concourse.tile as tile
from concourse._compat import get_trn_type
from concourse.bass_utils import run_bass_kernel_spmd

# Problem shape (hardcoded per contract)
B, P, H, W, C = 16, 22, 128, 128, 256
NUM_CLASSES = 10
N_CORES = 8
B_LOC = B // N_CORES        # 2 images per core
NROI = B_LOC * P            # 44 rois per core
NRS = 48                    # roi slots (8 partition-blocks x 6 groups)
NG = 6                      # roi-slot groups
HID1, HID2 = 128, 64
F32 = mybir.dt.float32
F16 = mybir.dt.float16
I32 = mybir.dt.int32
AX_X = mybir.AxisListType.X
OP = mybir.AluOpType
AF = mybir.ActivationFunctionType

HP = H - 1                      # 127 paired rows per image
NPROW = B_LOC * HP * W          # 32512 fmP pixel rows per core
MAX_PIX = NPROW - 2             # last valid fmP row start
MAGIC = 12582912.0              # 1.5 * 2^23 fp32 round-to-int magic
CHUNKS = [(0, 3), (3, 5), (5, 6)]   # combine/matmul chunk group-ranges
NCH = len(CHUNKS)


def _static_consts():
    ident = np.eye(128).astype(np.float16)
    # SEL[k, p] = 1 iff p//16 == k//2: rows 2h (hi) and 2h+1 (lo) of the
    # host-prepared prop_sm both route to partition block h.
    sel = np.zeros((16, 128), np.float16)
    for k in range(16):
        sel[k, (k // 2) * 16:(k // 2 + 1) * 16] = 1.0
    p = np.arange(128)
    q = p % 16
    cy = ((q // 4).astype(np.float32) + 0.5) / 4.0
    cx = ((q % 4).astype(np.float32) + 0.5) / 4.0
    h = (p // 16)[:, None]
    g = np.arange(NG)[None, :]
    roi = h * 6 + g
    bofs = np.where(roi >= P, float(HP * W), 0.0).astype(np.float32)  # [128,6]
    cb = np.concatenate([cy[:, None], cx[:, None], bofs], axis=1)
    idx0 = np.zeros((16, 1), np.int32)
    return ident, sel, np.ascontiguousarray(cb.astype(np.float32)), idx0


def emit_kernel(nc, tc, fm, prop, W1, b1, W2, b2, W3, b3, out, consts):
    """Emit the per-core tile kernel. All args are bass.APs."""
    with (
        tc.tile_pool(name="const", bufs=1) as cpool,
        tc.tile_pool(name="work", bufs=1) as wpool,
        tc.tile_pool(name="psum", bufs=1, space="PSUM") as ppool,
    ):
        _emit_body(nc, tc, fm, prop, W1, b1, W2, b2, W3, b3, out, consts,
                   cpool, wpool, ppool)


def _emit_mm1(nc, V, chunk, cix, svT, W1sb, psum1, l1v, b1sb):
    g0, g1 = chunk
    ng = g1 - g0
    for h in range(2):
        for q in range(16):
            k = q * 2 + h
            rhs = svT[h][:, g0 * 128:g1 * 128] \
                .rearrange("p (b a s) -> p a b s", b=ng, a=8)[:, :, :, q]
            nc.tensor.matmul(out=psum1[cix][:],
                             lhsT=W1sb[:, k * 128:(k + 1) * 128],
                             rhs=rhs, start=(h == 0 and q == 0),
                             stop=(h == 1 and q == 15))
    V.tensor_scalar(out=l1v[:, :, g0:g1], in0=psum1[cix][:],
                    scalar1=b1sb[:, 0:1], scalar2=0.0,
                    op0=OP.add, op1=OP.max)


def _emit_body(nc, tc, fm, prop, W1, b1, W2, b2, W3, b3, out, consts,
               cpool, wpool, ppool):
    ident_c, sel_c, cb_c, idx0_c_ap = consts
    V = nc.vector

    # ---------------- coords via selection matmul ----------------
    # psCB[p, (g, k)] = sum_k SEL[k, p] * prop_sm[k, (g, k4)] reconstructs
    # exact fp32 coords for partition block h = p//16 (hi+lo fp16 rows).
    phl = wpool.tile([16, 24], F16, name="phl")
    nc.sync.dma_start(phl[:], prop)
    sel = cpool.tile([16, 128], F16, name="sel")
    nc.sync.dma_start(sel[:], sel_c)
    cb = cpool.tile([128, 8], F32, name="cb")
    nc.sync.dma_start(cb[:], cb_c)
    idx0s = cpool.tile([16, 1], I32, name="idx0")
    nc.sync.dma_start(idx0s[:], idx0_c_ap)
    idx0_c = idx0s[:]
    ident = cpool.tile([128, 128], F16, name="ident")
    psCB = ppool.tile([128, 24], F32, name="psCB")
    nc.tensor.matmul(out=psCB[:], lhsT=sel[:], rhs=phl[:], start=True,
                     stop=True)
    CB = cpool.tile([128, 24], F32, name="CB")
    V.tensor_copy(out=CB[:], in_=psCB[:])

    # scalar queue: remaining loads; W1 (the big one) last.
    nc.scalar.dma_start(ident[:], ident_c)
    W2sb = cpool.tile([128, HID2], F16, name="W2sb")
    nc.scalar.dma_start(W2sb[:], W2)
    W3sb = cpool.tile([HID2, NUM_CLASSES], F16, name="W3sb")
    nc.scalar.dma_start(W3sb[:], W3)
    b1sb = cpool.tile([128, 1], F32, name="b1sb")
    nc.scalar.dma_start(b1sb[:], b1.rearrange("(p o) -> p o", o=1))
    b2sb = cpool.tile([HID2, 1], F32, name="b2sb")
    nc.scalar.dma_start(b2sb[:], b2.rearrange("(p o) -> p o", o=1))
    b3sb = cpool.tile([NROI, NUM_CLASSES], F32, name="b3sb")
    nc.scalar.dma_start(b3sb[:], b3.unsqueeze(0).to_broadcast([NROI, NUM_CLASSES]))
    W1sb = cpool.tile([128, 4096], F16, name="W1sb")
    nc.scalar.dma_start(W1sb[:], W1)

    # ---------------- index chain (critical path to the gathers) --------
    # Sample-major [128, (g, yx)] views.
    cgv = CB[:, :].rearrange("p (g k) -> p g k", g=NG)
    bofs = cb[:, 2:8]
    dyx = wpool.tile([128, 12], F32, name="dyx")
    syx = wpool.tile([128, 12], F32, name="syx")
    f0m = wpool.tile([128, 12], F32, name="f0m")
    f0 = wpool.tile([128, 12], F32, name="f0")
    pixb = wpool.tile([128, NG], F32, name="pixb")
    dv = dyx[:, :].rearrange("p (g yx) -> p g yx", yx=2)
    sv_ = syx[:, :].rearrange("p (g yx) -> p g yx", yx=2)
    f0v = f0[:, :].rearrange("p (g yx) -> p g yx", yx=2)
    cyx = cb[:, 0:2].unsqueeze(1).to_broadcast([128, NG, 2])

    V.tensor_tensor(out=dv[:], in0=cgv[:, :, 2:4], in1=cgv[:, :, 0:2],
                    op=OP.subtract)
    V.tensor_tensor(out=sv_[:], in0=dv[:], in1=cyx, op=OP.mult)
    V.tensor_tensor(out=sv_[:], in0=sv_[:], in1=cgv[:, :, 0:2], op=OP.add)
    # f0 = round(s - 0.5) via fp32 magic; consistent-pair bilinear stays exact
    V.tensor_scalar(out=f0m[:], in0=syx[:], scalar1=-0.5, scalar2=MAGIC,
                    op0=OP.add, op1=OP.add)
    V.tensor_scalar(out=f0[:], in0=f0m[:], scalar1=-MAGIC, scalar2=None,
                    op0=OP.add)
    # pixb = b*HP*W + y0*W + x0, clamped
    V.tensor_scalar(out=pixb[:], in0=f0v[:, :, 0], scalar1=float(W),
                    scalar2=None, op0=OP.mult)
    V.tensor_tensor(out=pixb[:], in0=pixb[:], in1=f0v[:, :, 1], op=OP.add)
    V.tensor_tensor(out=pixb[:], in0=pixb[:], in1=bofs, op=OP.add)
    V.tensor_scalar(out=pixb[:], in0=pixb[:], scalar1=0.0,
                    scalar2=float(MAX_PIX), op0=OP.max, op1=OP.min)
    idx = cpool.tile([128, NG], I32, name="gidx")
    V.tensor_copy(out=idx[:], in_=pixb[:])

    # warm up the SWDGE path so the first real gather doesn't pay ~1.2us
    # of cold-start: a tiny 16-descriptor indirect gather of row 0.
    warm = wpool.tile([128, 32], F16, name="warm")
    nc.gpsimd.indirect_dma_start(
        out=warm[0:16, :],
        out_offset=None,
        in_=fm.rearrange("b h w c -> (b h w) c"),
        in_offset=bass.IndirectOffsetOnAxis(ap=idx0_c, axis=0),
    )

    # ---------------- gathers: 6 indirect DMAs (128 descriptors) ---------
    # G[p, (g, x, ab, c)] fp16; each descriptor reads fmP rows idx, idx+1 =
    # pixels (x0, x0+1) x (row pair) x 256 ch. No gpsimd library needed.
    G = wpool.tile([128, NG * 1024], F16, name="gather")
    fmr = fm.rearrange("b h w c -> (b h w) c")            # [32512, 512]
    for j in range(NG):
        nc.gpsimd.indirect_dma_start(
            out=G[:, j * 1024:(j + 1) * 1024],
            out_offset=None,
            in_=fmr,
            in_offset=bass.IndirectOffsetOnAxis(ap=idx[:, j:j + 1], axis=0),
        )

    # ------- bilinear corner weights (off the gather critical path) -------
    lyx = wpool.tile([128, 12], F32, name="lyx")
    hyx = wpool.tile([128, 12], F32, name="hyx")
    V.tensor_tensor(out=lyx[:], in0=syx[:], in1=f0[:], op=OP.subtract)
    V.tensor_scalar(out=hyx[:], in0=lyx[:], scalar1=-1.0, scalar2=1.0,
                    op0=OP.mult, op1=OP.add)
    lv = lyx[:, :].rearrange("p (g yx) -> p g yx", yx=2)
    hv = hyx[:, :].rearrange("p (g yx) -> p g yx", yx=2)
    ly, lx = lv[:, :, 0], lv[:, :, 1]
    hy, hx = hv[:, :, 0], hv[:, :, 1]
    # wc[p, (g, x, ab)] fp16 (matches the fmP elem layout x-outer)
    wc = cpool.tile([128, 24], F16, name="wcat")
    wv = wc[:, :].rearrange("p (g x ab) -> p g x ab", x=2, ab=2)
    V.tensor_tensor(out=wv[:, :, 0, 0], in0=hy, in1=hx, op=OP.mult)
    V.tensor_tensor(out=wv[:, :, 0, 1], in0=ly, in1=hx, op=OP.mult)
    V.tensor_tensor(out=wv[:, :, 1, 0], in0=hy, in1=lx, op=OP.mult)
    V.tensor_tensor(out=wv[:, :, 1, 1], in0=ly, in1=lx, op=OP.mult)
    # wbig[cix]: wc chunk expanded over channels so the combine multiply
    # reads contiguous fp16 at full DVE rate (broadcast reads run ~2x slow).
    wbig = [wpool.tile([128, (g1 - g0) * 1024], F16, name=f"wbig{c}")
            for c, (g0, g1) in enumerate(CHUNKS)]
    for cix, (g0, g1) in enumerate(CHUNKS):
        ng = g1 - g0
        src = wc[:, g0 * 4:g1 * 4] \
            .rearrange("p (g x ab) -> p g x ab", x=2, ab=2).unsqueeze(4) \
            .to_broadcast([128, ng, 2, 2, C])
        dst = wbig[cix][:, :].rearrange("p (g x ab c) -> p g x ab c",
                                        g=ng, x=2, ab=2)
        if cix % 2 == 0:
            V.tensor_copy(out=dst, in_=src)
        else:
            nc.scalar.copy(out=dst, in_=src)

    # ---------------- bilinear combine + transpose, per 2-group chunk -----
    Gv = G[:, :].rearrange("p (g x ab c) -> p g x ab c", g=NG, x=2, ab=2)
    sv2 = wpool.tile([128, NG * 512], F16, name="sv2")
    sv = wpool.tile([128, NG * 256], F16, name="sv")
    s2v = sv2[:, :].rearrange("p (g x c) -> p g x c", g=NG, x=2)
    svv = sv[:, :].rearrange("p (g c) -> p g c", g=NG)
    svT = [wpool.tile([128, NG * 128], F16, name=f"svT{h}") for h in range(2)]
    # layer-1 psum per chunk: columns (a, b_chunk); l1 interleaves to (a, b)
    psum1 = [ppool.tile([128, 8 * (g1 - g0)], F32, name=f"psum1{c}")
             for c, (g0, g1) in enumerate(CHUNKS)]
    l1 = wpool.tile([128, NRS], F16, name="l1")
    l1v = l1[:, :].rearrange("p (a b) -> p a b", a=8)
    for cix, (g0, g1) in enumerate(CHUNKS):
        ng = g1 - g0
        gs = slice(g0, g1)
        V.tensor_tensor(out=Gv[:, gs], in0=Gv[:, gs],
                        in1=wbig[cix][:, :].rearrange(
                            "p (g x ab c) -> p g x ab c", g=ng, x=2, ab=2),
                        op=OP.mult)
        V.tensor_tensor(out=s2v[:, gs], in0=Gv[:, gs, :, 0], in1=Gv[:, gs, :, 1],
                        op=OP.add)
        V.tensor_tensor(out=svv[:, gs], in0=s2v[:, gs, 0], in1=s2v[:, gs, 1],
                        op=OP.add)
        for g in range(g0, g1):
            for h in range(2):
                pt = ppool.tile([128, 128], F16, tag="pt", bufs=3, name="pt")
                nc.tensor.transpose(
                    out=pt[:],
                    in_=sv[:, g * 256 + h * 128: g * 256 + (h + 1) * 128],
                    identity=ident[:])
                nc.scalar.copy(out=svT[h][:, g * 128:(g + 1) * 128],
                               in_=pt[:])
        # layer-1 matmuls for the PREVIOUS chunk (one-chunk skew keeps the
        # PE queue's transposes for this chunk ahead of bulk matmul work)
        if cix > 0:
            _emit_mm1(nc, V, CHUNKS[cix - 1], cix - 1, svT, W1sb, psum1,
                      l1v, b1sb)
    _emit_mm1(nc, V, CHUNKS[NCH - 1], NCH - 1, svT, W1sb, psum1, l1v, b1sb)

    # ---------------- MLP layers 2, 3 ----------------
    psum23 = ppool.tile([HID2, NRS + NUM_CLASSES], F32, name="psum23")
    psum2 = psum23[:, 0:NRS]
    psum3 = psum23[0:NRS, NRS:NRS + NUM_CLASSES]
    nc.tensor.matmul(out=psum2, lhsT=W2sb[:, :], rhs=l1[:], start=True,
                     stop=True)
    l2 = wpool.tile([HID2, NRS], F16, name="l2")
    V.tensor_scalar(out=l2[:], in0=psum2, scalar1=b2sb[:, 0:1], scalar2=0.0,
                    op0=OP.add, op1=OP.max)
    nc.tensor.matmul(out=psum3, lhsT=l2[:], rhs=W3sb[:], start=True,
                     stop=True)

    # ---------------- softmax (rows 0..43 only, fp32) ----------------
    # logits are O(10), so fp32 exp needs no max-subtraction.
    logits = wpool.tile([NROI, NUM_CLASSES], F32, name="logits")
    V.tensor_tensor(out=logits[:], in0=psum3[0:NROI, :], in1=b3sb[:], op=OP.add)
    ex = wpool.tile([NROI, NUM_CLASSES], F32, name="ex")
    nc.scalar.activation(out=ex[:], in_=logits[:], func=AF.Exp,
                         bias=0.0, scale=1.0)
    ssum = wpool.tile([NROI, 1], F32, name="ssum")
    V.tensor_reduce(out=ssum[:], in_=ex[:], axis=AX_X, op=OP.add)
    rinv = wpool.tile([NROI, 1], F32, name="rinv")
    V.reciprocal(rinv[:], ssum[:])
    probs = wpool.tile([NROI, NUM_CLASSES], F32, name="probs")
    V.tensor_scalar(out=probs[:], in0=ex[:], scalar1=rinv[:, 0:1],
                    scalar2=None, op0=OP.mult)

    nc.sync.dma_start(out.rearrange("b p c -> (b p) c"), probs[:])


def build_module():
    nc = bacc.Bacc(get_trn_type() or "TRN2", target_bir_lowering=False, debug=False)
    fm = nc.dram_tensor("feature_map", [B_LOC, HP, W, 2 * C], F16, kind="ExternalInput")
    prop = nc.dram_tensor("proposals", [16, 24], F16, kind="ExternalInput")
    W1 = nc.dram_tensor("W1", [128, 4096], F16, kind="ExternalInput")
    b1 = nc.dram_tensor("b1", [HID1], F32, kind="ExternalInput")
    W2 = nc.dram_tensor("W2", [HID1, HID2], F16, kind="ExternalInput")
    b2 = nc.dram_tensor("b2", [HID2], F32, kind="ExternalInput")
    W3 = nc.dram_tensor("W3", [HID2, NUM_CLASSES], F16, kind="ExternalInput")
    b3 = nc.dram_tensor("b3", [NUM_CLASSES], F32, kind="ExternalInput")
    out = nc.dram_tensor("out", [B_LOC, P, NUM_CLASSES], F32, kind="ExternalOutput")

    ident_np, sel_np, cb_np, idx0_np = _static_consts()
    ident_c = nc.inline_tensor(ident_np, name="c_ident")
    sel_c = nc.inline_tensor(sel_np, name="c_sel")
    cb_c = nc.inline_tensor(cb_np, name="c_cb")
    idx0_c = nc.inline_tensor(idx0_np, name="c_idx0")

    with tile.TileContext(nc) as tc:
        emit_kernel(nc, tc, fm[:], prop[:], W1[:], b1[:], W2[:], b2[:], W3[:],
                    b3[:], out[:], (ident_c[:], sel_c[:], cb_c[:], idx0_c[:]))
    nc.compile()
    return nc


_NC_CACHE = None


def _get_module():
    global _NC_CACHE
    if _NC_CACHE is None:
        _NC_CACHE = build_module()
    return _NC_CACHE


def _shard_inputs(inputs):
    fm16 = np.asarray(inputs["feature_map"], dtype=np.float32).astype(np.float16)
    # paired rows: fmP[b, y, x] = fm[b, y] ++ fm[b, y+1] per pixel
    fmP = np.concatenate([fm16[:, :-1], fm16[:, 1:]], axis=3)
    fmP = np.ascontiguousarray(fmP)
    props = np.asarray(inputs["proposals"], dtype=np.float32)
    # W1 rows k*128+p -> [p, k*128+j] fp16 so lhsT chunks are contiguous.
    W1h = np.ascontiguousarray(
        np.asarray(inputs["W1"], dtype=np.float32).reshape(32, 128, HID1)
        .transpose(1, 0, 2).reshape(128, 4096).astype(np.float16))
    W2h = np.ascontiguousarray(np.asarray(inputs["W2"], dtype=np.float32).astype(np.float16))
    W3h = np.ascontiguousarray(np.asarray(inputs["W3"], dtype=np.float32).astype(np.float16))
    b1h = np.ascontiguousarray(np.asarray(inputs["b1"], dtype=np.float32))
    b2h = np.ascontiguousarray(np.asarray(inputs["b2"], dtype=np.float32))
    b3h = np.ascontiguousarray(np.asarray(inputs["b3"], dtype=np.float32))
    in_maps = []
    for c in range(N_CORES):
        sl = slice(B_LOC * c, B_LOC * (c + 1))
        # prop_sm[2h]   = fp16 hi of coords for roi slots (h, g=0..5)
        # prop_sm[2h+1] = fp16 lo; h=7 slots hold rois [42, 43, 38..41].
        pf = props[sl].reshape(NROI, 4)
        rows = np.zeros((8, 6, 4), np.float32)
        for h in range(7):
            rows[h] = pf[h * 6:(h + 1) * 6]
        rows[7, 0:2] = pf[42:44]
        rows[7, 2:6] = pf[38:42]
        rows = rows.reshape(8, 24)
        hi = rows.astype(np.float16)
        lo = (rows - hi.astype(np.float32)).astype(np.float16)
        phl = np.zeros((16, 24), np.float16)
        phl[0::2] = hi
        phl[1::2] = lo
        in_maps.append({
            "feature_map": fmP[sl],
            "proposals": np.ascontiguousarray(phl),
            "W1": W1h, "b1": b1h,
            "W2": W2h, "b2": b2h,
            "W3": W3h, "b3": b3h,
        })
    return in_maps


def run(inputs, trace=False):
    """Run on all 8 cores; returns (output [16,22,10], BassKernelResults)."""
    nc = _get_module()
    res = run_bass_kernel_spmd(nc, _shard_inputs(inputs), core_ids=list(range(N_CORES)),
                               trace=trace)
    out = np.concatenate([r["out"] for r in res.results], axis=0)
    return out, res


def kernel(**inputs) -> np.ndarray:
    out, _ = run(inputs, trace=False)
    return out


# revision 30
# speedup vs baseline: 1.0615x; 1.0454x over previous
"""ROI-Align + MLP classification head (nms_detection) on 8 Trainium2 cores.

Strategy: data-parallel over batch (2 images per core). Host pre-casts the
feature map to fp16 and stores it row-paired (fmP[b, y, x] = fm[b, y, x] ++
fm[b, y+1, x], 512 ch), so ONE 2KB gather descriptor fetches all 4 bilinear
corners of a sample. MLP weights are pre-cast/arranged fp16 on host; the
proposals ship as an fp16 hi/lo split laid out per roi-slot block so a
single K=16 selection matmul (SEL.T @ prop_rows) materializes exact fp32
sample-major coords on all 128 partitions. The index chain runs on the
vector engine ([128, 6] tiles), six indirect DMAs (128 descriptors each, no
gpsimd library needed) fetch the sample blocks, and the bilinear combine
(fp16, weights pre-expanded over channels for full DVE rate) + PE transpose
+ 3-layer MLP (fp16 in / fp32 psum) + fp32 softmax finish, pipelined per
2-group chunk.

Layouts (per core): 44 rois x 16 bin-centers = 704 samples.
  roi slot (h, g): roi = h*6 + g, h in 0..7, g in 0..5 (48 slots, 4 garbage)
  sample partition p = h*16 + q (q = iy*4+ix); gather block j = g (6 blocks)
  idx value = fmP row = b*(H-1)*W + y0*W + x0 (int32); each gather reads
  rows idx..idx+1 = pixels (x0, x0+1) x (row pair y0, y0+1) x 256 ch.
"""

import numpy as np

import concourse.bacc as bacc
import concourse.bass as bass
import concourse.mybir as mybir
import concourse.tile as tile
from concourse._compat import get_trn_type
from concourse.bass_utils import run_bass_kernel_spmd

# Problem shape (hardcoded per contract)
B, P, H, W, C = 16, 22, 128, 128, 256
NUM_CLASSES = 10
N_CORES = 8
B_LOC = B // N_CORES        # 2 images per core
NROI = B_LOC * P            # 44 rois per core
NRS = 48                    # roi slots (8 partition-blocks x 6 groups)
NG = 6                      # roi-slot groups
HID1, HID2 = 128, 64
F32 = mybir.dt.float32
F16 = mybir.dt.float16
I32 = mybir.dt.int32
AX_X = mybir.AxisListType.X
OP = mybir.AluOpType
AF = mybir.ActivationFunctionType

HP = H - 1                      # 127 paired rows per image
NPROW = B_LOC * HP * W          # 32512 fmP pixel rows per core
MAX_PIX = NPROW - 2             # last valid fmP row start
MAGIC = 12582912.0              # 1.5 * 2^23 fp32 round-to-int magic
CHUNKS = [(0, 2), (2, 4), (4, 6)]   # matmul psum chunk group-ranges
NCH = len(CHUNKS)


def _static_consts():
    ident = np.eye(128).astype(np.float16)
    # SEL[k, p] = 1 iff p//16 == k//2: rows 2h (hi) and 2h+1 (lo) of the
    # host-prepared prop_sm both route to partition block h.
    sel = np.zeros((16, 128), np.float16)
    for k in range(16):
        sel[k, (k // 2) * 16:(k // 2 + 1) * 16] = 1.0
    p = np.arange(128)
    q = p % 16
    cy = ((q // 4).astype(np.float32) + 0.5) / 4.0
    cx = ((q % 4).astype(np.float32) + 0.5) / 4.0
    h = (p // 16)[:, None]
    g = np.arange(NG)[None, :]
    roi = h * 6 + g
    bofs = np.where(roi >= P, float(HP * W), 0.0).astype(np.float32)  # [128,6]
    cb = np.concatenate([cy[:, None], cx[:, None], bofs], axis=1)
    idx0 = np.zeros((16, 1), np.int32)
    return ident, sel, np.ascontiguousarray(cb.astype(np.float32)), idx0


def emit_kernel(nc, tc, fm, prop, W1, b1, W2, b2, W3, b3, out, consts):
    """Emit the per-core tile kernel. All args are bass.APs."""
    with (
        tc.tile_pool(name="const", bufs=1) as cpool,
        tc.tile_pool(name="work", bufs=1) as wpool,
        tc.tile_pool(name="psum", bufs=1, space="PSUM") as ppool,
    ):
        _emit_body(nc, tc, fm, prop, W1, b1, W2, b2, W3, b3, out, consts,
                   cpool, wpool, ppool)


def _emit_mm1(nc, V, chunk, cix, svT, W1sb, psum1, l1v, b1sb):
    g0, g1 = chunk
    ng = g1 - g0
    for h in range(2):
        for q in range(16):
            k = q * 2 + h
            rhs = svT[h][:, g0 * 128:g1 * 128] \
                .rearrange("p (b a s) -> p a b s", b=ng, a=8)[:, :, :, q]
            nc.tensor.matmul(out=psum1[cix][:],
                             lhsT=W1sb[:, k * 128:(k + 1) * 128],
                             rhs=rhs, start=(h == 0 and q == 0),
                             stop=(h == 1 and q == 15))
    V.tensor_scalar(out=l1v[:, :, g0:g1], in0=psum1[cix][:],
                    scalar1=b1sb[:, 0:1], scalar2=0.0,
                    op0=OP.add, op1=OP.max)


def _emit_body(nc, tc, fm, prop, W1, b1, W2, b2, W3, b3, out, consts,
               cpool, wpool, ppool):
    ident_c, sel_c, cb_c, idx0_c_ap = consts
    V = nc.vector

    # ---------------- coords via selection matmul ----------------
    # psCB[p, (g, k)] = sum_k SEL[k, p] * prop_sm[k, (g, k4)] reconstructs
    # exact fp32 coords for partition block h = p//16 (hi+lo fp16 rows).
    phl = wpool.tile([16, 24], F16, name="phl")
    nc.sync.dma_start(phl[:], prop)
    sel = cpool.tile([16, 128], F16, name="sel")
    nc.sync.dma_start(sel[:], sel_c)
    cb = cpool.tile([128, 8], F32, name="cb")
    nc.sync.dma_start(cb[:], cb_c)
    idx0s = cpool.tile([16, 1], I32, name="idx0")
    nc.sync.dma_start(idx0s[:], idx0_c_ap)
    idx0_c = idx0s[:]
    ident = cpool.tile([128, 128], F16, name="ident")
    psCB = ppool.tile([128, 24], F32, name="psCB")
    nc.tensor.matmul(out=psCB[:], lhsT=sel[:], rhs=phl[:], start=True,
                     stop=True)
    CB = cpool.tile([128, 24], F32, name="CB")
    V.tensor_copy(out=CB[:], in_=psCB[:])

    # scalar queue: remaining loads; W1 (the big one) last.
    nc.scalar.dma_start(ident[:], ident_c)
    W2sb = cpool.tile([128, HID2], F16, name="W2sb")
    nc.scalar.dma_start(W2sb[:], W2)
    W3sb = cpool.tile([HID2, NUM_CLASSES], F16, name="W3sb")
    nc.scalar.dma_start(W3sb[:], W3)
    b1sb = cpool.tile([128, 1], F32, name="b1sb")
    nc.scalar.dma_start(b1sb[:], b1.rearrange("(p o) -> p o", o=1))
    b2sb = cpool.tile([HID2, 1], F32, name="b2sb")
    nc.scalar.dma_start(b2sb[:], b2.rearrange("(p o) -> p o", o=1))
    b3sb = cpool.tile([NROI, NUM_CLASSES], F32, name="b3sb")
    nc.scalar.dma_start(b3sb[:], b3.unsqueeze(0).to_broadcast([NROI, NUM_CLASSES]))
    W1sb = cpool.tile([128, 4096], F16, name="W1sb")
    nc.scalar.dma_start(W1sb[:], W1)

    # ---------------- index chain (critical path to the gathers) --------
    # Sample-major [128, (g, yx)] views.
    cgv = CB[:, :].rearrange("p (g k) -> p g k", g=NG)
    bofs = cb[:, 2:8]
    dyx = wpool.tile([128, 12], F32, name="dyx")
    syx = wpool.tile([128, 12], F32, name="syx")
    f0m = wpool.tile([128, 12], F32, name="f0m")
    f0 = wpool.tile([128, 12], F32, name="f0")
    pixb = wpool.tile([128, NG], F32, name="pixb")
    dv = dyx[:, :].rearrange("p (g yx) -> p g yx", yx=2)
    sv_ = syx[:, :].rearrange("p (g yx) -> p g yx", yx=2)
    f0v = f0[:, :].rearrange("p (g yx) -> p g yx", yx=2)
    cyx = cb[:, 0:2].unsqueeze(1).to_broadcast([128, NG, 2])

    V.tensor_tensor(out=dv[:], in0=cgv[:, :, 2:4], in1=cgv[:, :, 0:2],
                    op=OP.subtract)
    V.tensor_tensor(out=sv_[:], in0=dv[:], in1=cyx, op=OP.mult)
    V.tensor_tensor(out=sv_[:], in0=sv_[:], in1=cgv[:, :, 0:2], op=OP.add)
    # f0 = round(s - 0.5) via fp32 magic; consistent-pair bilinear stays exact
    V.tensor_scalar(out=f0m[:], in0=syx[:], scalar1=-0.5, scalar2=MAGIC,
                    op0=OP.add, op1=OP.add)
    V.tensor_scalar(out=f0[:], in0=f0m[:], scalar1=-MAGIC, scalar2=None,
                    op0=OP.add)
    # pixb = b*HP*W + y0*W + x0, clamped
    V.tensor_scalar(out=pixb[:], in0=f0v[:, :, 0], scalar1=float(W),
                    scalar2=None, op0=OP.mult)
    V.tensor_tensor(out=pixb[:], in0=pixb[:], in1=f0v[:, :, 1], op=OP.add)
    V.tensor_tensor(out=pixb[:], in0=pixb[:], in1=bofs, op=OP.add)
    V.tensor_scalar(out=pixb[:], in0=pixb[:], scalar1=0.0,
                    scalar2=float(MAX_PIX), op0=OP.max, op1=OP.min)
    idx = cpool.tile([128, NG], I32, name="gidx")
    V.tensor_copy(out=idx[:], in_=pixb[:])

    # warm up the SWDGE path so the first real gather doesn't pay ~1.2us
    # of cold-start: a tiny 16-descriptor indirect gather of row 0.
    warm = wpool.tile([128, 32], F16, name="warm")
    nc.gpsimd.indirect_dma_start(
        out=warm[0:16, :],
        out_offset=None,
        in_=fm.rearrange("b h w c -> (b h w) c"),
        in_offset=bass.IndirectOffsetOnAxis(ap=idx0_c, axis=0),
    )

    # ---------------- gathers: 6 indirect DMAs (128 descriptors) ---------
    # G[p, (g, x, ab, c)] fp16; each descriptor reads fmP rows idx, idx+1 =
    # pixels (x0, x0+1) x (row pair) x 256 ch. No gpsimd library needed.
    G = wpool.tile([128, NG * 1024], F16, name="gather")
    fmr = fm.rearrange("b h w c -> (b h w) c")            # [32512, 512]
    for j in range(NG):
        nc.gpsimd.indirect_dma_start(
            out=G[:, j * 1024:(j + 1) * 1024],
            out_offset=None,
            in_=fmr,
            in_offset=bass.IndirectOffsetOnAxis(ap=idx[:, j:j + 1], axis=0),
        )

    # ------- bilinear corner weights (off the gather critical path) -------
    lyx = wpool.tile([128, 12], F32, name="lyx")
    hyx = wpool.tile([128, 12], F32, name="hyx")
    V.tensor_tensor(out=lyx[:], in0=syx[:], in1=f0[:], op=OP.subtract)
    V.tensor_scalar(out=hyx[:], in0=lyx[:], scalar1=-1.0, scalar2=1.0,
                    op0=OP.mult, op1=OP.add)
    lv = lyx[:, :].rearrange("p (g yx) -> p g yx", yx=2)
    hv = hyx[:, :].rearrange("p (g yx) -> p g yx", yx=2)
    ly, lx = lv[:, :, 0], lv[:, :, 1]
    hy, hx = hv[:, :, 0], hv[:, :, 1]
    # wc[p, (g, x, ab)] fp16 (matches the fmP elem layout x-outer)
    wc = cpool.tile([128, 24], F16, name="wcat")
    wv = wc[:, :].rearrange("p (g x ab) -> p g x ab", x=2, ab=2)
    V.tensor_tensor(out=wv[:, :, 0, 0], in0=hy, in1=hx, op=OP.mult)
    V.tensor_tensor(out=wv[:, :, 0, 1], in0=ly, in1=hx, op=OP.mult)
    V.tensor_tensor(out=wv[:, :, 1, 0], in0=hy, in1=lx, op=OP.mult)
    V.tensor_tensor(out=wv[:, :, 1, 1], in0=ly, in1=lx, op=OP.mult)
    # wbig[cix]: wc chunk expanded over channels so the combine multiply
    # reads contiguous fp16 at full DVE rate (broadcast reads run ~2x slow).
    wbig = [wpool.tile([128, (g1 - g0) * 1024], F16, name=f"wbig{c}")
            for c, (g0, g1) in enumerate(CHUNKS)]
    for cix, (g0, g1) in enumerate(CHUNKS):
        ng = g1 - g0
        src = wc[:, g0 * 4:g1 * 4] \
            .rearrange("p (g x ab) -> p g x ab", x=2, ab=2).unsqueeze(4) \
            .to_broadcast([128, ng, 2, 2, C])
        dst = wbig[cix][:, :].rearrange("p (g x ab c) -> p g x ab c",
                                        g=ng, x=2, ab=2)
        if cix % 2 == 0:
            V.tensor_copy(out=dst, in_=src)
        else:
            nc.scalar.copy(out=dst, in_=src)

    # ---------------- bilinear combine + transpose, per 2-group chunk -----
    Gv = G[:, :].rearrange("p (g x ab c) -> p g x ab c", g=NG, x=2, ab=2)
    sv2 = wpool.tile([128, NG * 512], F16, name="sv2")
    sv = wpool.tile([128, NG * 256], F16, name="sv")
    s2v = sv2[:, :].rearrange("p (g x c) -> p g x c", g=NG, x=2)
    svv = sv[:, :].rearrange("p (g c) -> p g c", g=NG)
    svT = [wpool.tile([128, NG * 128], F16, name=f"svT{h}") for h in range(2)]
    # layer-1 psum per chunk: columns (a, b_chunk); l1 interleaves to (a, b)
    psum1 = [ppool.tile([128, 8 * (g1 - g0)], F32, name=f"psum1{c}")
             for c, (g0, g1) in enumerate(CHUNKS)]
    l1 = wpool.tile([128, NRS], F16, name="l1")
    l1v = l1[:, :].rearrange("p (a b) -> p a b", a=8)
    # combine + transpose per GROUP as each gather lands; matmuls per pair
    for g in range(NG):
        cix, g0 = g // 2, (g // 2) * 2
        gs = slice(g, g + 1)
        wco = (g - g0) * 1024
        V.tensor_tensor(out=Gv[:, gs], in0=Gv[:, gs],
                        in1=wbig[cix][:, wco:wco + 1024].rearrange(
                            "p (g x ab c) -> p g x ab c", g=1, x=2, ab=2),
                        op=OP.mult)
        V.tensor_tensor(out=s2v[:, gs], in0=Gv[:, gs, :, 0], in1=Gv[:, gs, :, 1],
                        op=OP.add)
        V.tensor_tensor(out=svv[:, gs], in0=s2v[:, gs, 0], in1=s2v[:, gs, 1],
                        op=OP.add)
        for h in range(2):
            pt = ppool.tile([128, 128], F16, tag="pt", bufs=3, name="pt")
            nc.tensor.transpose(
                out=pt[:],
                in_=sv[:, g * 256 + h * 128: g * 256 + (h + 1) * 128],
                identity=ident[:])
            nc.scalar.copy(out=svT[h][:, g * 128:(g + 1) * 128], in_=pt[:])
        if g % 2 == 1:
            _emit_mm1(nc, V, CHUNKS[cix], cix, svT, W1sb, psum1, l1v, b1sb)

    # ---------------- MLP layers 2, 3 ----------------
    psum23 = ppool.tile([HID2, NRS + NUM_CLASSES], F32, name="psum23")
    psum2 = psum23[:, 0:NRS]
    psum3 = psum23[0:NRS, NRS:NRS + NUM_CLASSES]
    nc.tensor.matmul(out=psum2, lhsT=W2sb[:, :], rhs=l1[:], start=True,
                     stop=True)
    l2 = wpool.tile([HID2, NRS], F16, name="l2")
    V.tensor_scalar(out=l2[:], in0=psum2, scalar1=b2sb[:, 0:1], scalar2=0.0,
                    op0=OP.add, op1=OP.max)
    nc.tensor.matmul(out=psum3, lhsT=l2[:], rhs=W3sb[:], start=True,
                     stop=True)

    # ---------------- softmax (rows 0..43 only, fp32) ----------------
    # logits are O(10), so fp32 exp needs no max-subtraction.
    logits = wpool.tile([NROI, NUM_CLASSES], F32, name="logits")
    V.tensor_tensor(out=logits[:], in0=psum3[0:NROI, :], in1=b3sb[:], op=OP.add)
    ex = wpool.tile([NROI, NUM_CLASSES], F32, name="ex")
    nc.scalar.activation(out=ex[:], in_=logits[:], func=AF.Exp,
                         bias=0.0, scale=1.0)
    ssum = wpool.tile([NROI, 1], F32, name="ssum")
    V.tensor_reduce(out=ssum[:], in_=ex[:], axis=AX_X, op=OP.add)
    rinv = wpool.tile([NROI, 1], F32, name="rinv")
    V.reciprocal(rinv[:], ssum[:])
    probs = wpool.tile([NROI, NUM_CLASSES], F32, name="probs")
    V.tensor_scalar(out=probs[:], in0=ex[:], scalar1=rinv[:, 0:1],
                    scalar2=None, op0=OP.mult)

    nc.sync.dma_start(out.rearrange("b p c -> (b p) c"), probs[:])


def build_module():
    nc = bacc.Bacc(get_trn_type() or "TRN2", target_bir_lowering=False, debug=False)
    fm = nc.dram_tensor("feature_map", [B_LOC, HP, W, 2 * C], F16, kind="ExternalInput")
    prop = nc.dram_tensor("proposals", [16, 24], F16, kind="ExternalInput")
    W1 = nc.dram_tensor("W1", [128, 4096], F16, kind="ExternalInput")
    b1 = nc.dram_tensor("b1", [HID1], F32, kind="ExternalInput")
    W2 = nc.dram_tensor("W2", [HID1, HID2], F16, kind="ExternalInput")
    b2 = nc.dram_tensor("b2", [HID2], F32, kind="ExternalInput")
    W3 = nc.dram_tensor("W3", [HID2, NUM_CLASSES], F16, kind="ExternalInput")
    b3 = nc.dram_tensor("b3", [NUM_CLASSES], F32, kind="ExternalInput")
    out = nc.dram_tensor("out", [B_LOC, P, NUM_CLASSES], F32, kind="ExternalOutput")

    ident_np, sel_np, cb_np, idx0_np = _static_consts()
    ident_c = nc.inline_tensor(ident_np, name="c_ident")
    sel_c = nc.inline_tensor(sel_np, name="c_sel")
    cb_c = nc.inline_tensor(cb_np, name="c_cb")
    idx0_c = nc.inline_tensor(idx0_np, name="c_idx0")

    with tile.TileContext(nc) as tc:
        emit_kernel(nc, tc, fm[:], prop[:], W1[:], b1[:], W2[:], b2[:], W3[:],
                    b3[:], out[:], (ident_c[:], sel_c[:], cb_c[:], idx0_c[:]))
    nc.compile()
    return nc


_NC_CACHE = None


def _get_module():
    global _NC_CACHE
    if _NC_CACHE is None:
        _NC_CACHE = build_module()
    return _NC_CACHE


def _shard_inputs(inputs):
    fm16 = np.asarray(inputs["feature_map"], dtype=np.float32).astype(np.float16)
    # paired rows: fmP[b, y, x] = fm[b, y] ++ fm[b, y+1] per pixel
    fmP = np.concatenate([fm16[:, :-1], fm16[:, 1:]], axis=3)
    fmP = np.ascontiguousarray(fmP)
    props = np.asarray(inputs["proposals"], dtype=np.float32)
    # W1 rows k*128+p -> [p, k*128+j] fp16 so lhsT chunks are contiguous.
    W1h = np.ascontiguousarray(
        np.asarray(inputs["W1"], dtype=np.float32).reshape(32, 128, HID1)
        .transpose(1, 0, 2).reshape(128, 4096).astype(np.float16))
    W2h = np.ascontiguousarray(np.asarray(inputs["W2"], dtype=np.float32).astype(np.float16))
    W3h = np.ascontiguousarray(np.asarray(inputs["W3"], dtype=np.float32).astype(np.float16))
    b1h = np.ascontiguousarray(np.asarray(inputs["b1"], dtype=np.float32))
    b2h = np.ascontiguousarray(np.asarray(inputs["b2"], dtype=np.float32))
    b3h = np.ascontiguousarray(np.asarray(inputs["b3"], dtype=np.float32))
    in_maps = []
    for c in range(N_CORES):
        sl = slice(B_LOC * c, B_LOC * (c + 1))
        # prop_sm[2h]   = fp16 hi of coords for roi slots (h, g=0..5)
        # prop_sm[2h+1] = fp16 lo; h=7 slots hold rois [42, 43, 38..41].
        pf = props[sl].reshape(NROI, 4)
        rows = np.zeros((8, 6, 4), np.float32)
        for h in range(7):
            rows[h] = pf[h * 6:(h + 1) * 6]
        rows[7, 0:2] = pf[42:44]
        rows[7, 2:6] = pf[38:42]
        rows = rows.reshape(8, 24)
        hi = rows.astype(np.float16)
        lo = (rows - hi.astype(np.float32)).astype(np.float16)
        phl = np.zeros((16, 24), np.float16)
        phl[0::2] = hi
        phl[1::2] = lo
        in_maps.append({
            "feature_map": fmP[sl],
            "proposals": np.ascontiguousarray(phl),
            "W1": W1h, "b1": b1h,
            "W2": W2h, "b2": b2h,
            "W3": W3h, "b3": b3h,
        })
    return in_maps


def run(inputs, trace=False):
    """Run on all 8 cores; returns (output [16,22,10], BassKernelResults)."""
    nc = _get_module()
    res = run_bass_kernel_spmd(nc, _shard_inputs(inputs), core_ids=list(range(N_CORES)),
                               trace=trace)
    out = np.concatenate([r["out"] for r in res.results], axis=0)
    return out, res


def kernel(**inputs) -> np.ndarray:
    out, _ = run(inputs, trace=False)
    return out


# revision 32
# speedup vs baseline: 1.0740x; 1.0118x over previous
"""ROI-Align + MLP classification head (nms_detection) on 8 Trainium2 cores.

Strategy: data-parallel over batch (2 images per core). Host pre-casts the
feature map to fp16 and stores it row-paired (fmP[b, y, x] = fm[b, y, x] ++
fm[b, y+1, x], 512 ch), so ONE 2KB gather descriptor fetches all 4 bilinear
corners of a sample. MLP weights are pre-cast/arranged fp16 on host; the
proposals ship as an fp16 hi/lo split laid out per roi-slot block so a
single K=16 selection matmul (SEL.T @ prop_rows) materializes exact fp32
sample-major coords on all 128 partitions. The index chain runs on the
vector engine ([128, 6] tiles), six indirect DMAs (128 descriptors each, no
gpsimd library needed) fetch the sample blocks, and the bilinear combine
(fp16, weights pre-expanded over channels for full DVE rate) + PE transpose
+ 3-layer MLP (fp16 in / fp32 psum) + fp32 softmax finish, pipelined per
2-group chunk.

Layouts (per core): 44 rois x 16 bin-centers = 704 samples.
  roi slot (h, g): roi = h*6 + g, h in 0..7, g in 0..5 (48 slots, 4 garbage)
  sample partition p = h*16 + q (q = iy*4+ix); gather block j = g (6 blocks)
  idx value = fmP row = b*(H-1)*W + y0*W + x0 (int32); each gather reads
  rows idx..idx+1 = pixels (x0, x0+1) x (row pair y0, y0+1) x 256 ch.
"""

import numpy as np

import concourse.bacc as bacc
import concourse.bass as bass
import concourse.mybir as mybir
import concourse.tile as tile
from concourse._compat import get_trn_type
from concourse.bass_utils import run_bass_kernel_spmd

# Problem shape (hardcoded per contract)
B, P, H, W, C = 16, 22, 128, 128, 256
NUM_CLASSES = 10
N_CORES = 8
B_LOC = B // N_CORES        # 2 images per core
NROI = B_LOC * P            # 44 rois per core
NRS = 48                    # roi slots (8 partition-blocks x 6 groups)
NG = 6                      # roi-slot groups
HID1, HID2 = 128, 64
F32 = mybir.dt.float32
F16 = mybir.dt.float16
I32 = mybir.dt.int32
AX_X = mybir.AxisListType.X
OP = mybir.AluOpType
AF = mybir.ActivationFunctionType

HP = H - 1                      # 127 paired rows per image
NPROW = B_LOC * HP * W          # 32512 fmP pixel rows per core
MAX_PIX = NPROW - 2             # last valid fmP row start
MAGIC = 12582912.0              # 1.5 * 2^23 fp32 round-to-int magic
CHUNKS = [(0, 2), (2, 4), (4, 6)]   # matmul psum chunk group-ranges
NCH = len(CHUNKS)


def _static_consts():
    ident = np.eye(128).astype(np.float16)
    # SEL[k, p] = 1 iff p//16 == k//2: rows 2h (hi) and 2h+1 (lo) of the
    # host-prepared prop_sm both route to partition block h.
    sel = np.zeros((16, 128), np.float16)
    for k in range(16):
        sel[k, (k // 2) * 16:(k // 2 + 1) * 16] = 1.0
    p = np.arange(128)
    q = p % 16
    cy = ((q // 4).astype(np.float32) + 0.5) / 4.0
    cx = ((q % 4).astype(np.float32) + 0.5) / 4.0
    h = (p // 16)[:, None]
    g = np.arange(NG)[None, :]
    roi = h * 6 + g
    bofs = np.where(roi >= P, float(HP * W), 0.0).astype(np.float32)  # [128,6]
    cb = np.concatenate([cy[:, None], cx[:, None], bofs], axis=1)
    idx0 = np.zeros((16, 1), np.int32)
    return ident, sel, np.ascontiguousarray(cb.astype(np.float32)), idx0


def emit_kernel(nc, tc, fm, prop, W1, b1, W2, b2, W3, b3, out, consts):
    """Emit the per-core tile kernel. All args are bass.APs."""
    with (
        tc.tile_pool(name="const", bufs=1) as cpool,
        tc.tile_pool(name="work", bufs=1) as wpool,
        tc.tile_pool(name="psum", bufs=1, space="PSUM") as ppool,
    ):
        _emit_body(nc, tc, fm, prop, W1, b1, W2, b2, W3, b3, out, consts,
                   cpool, wpool, ppool)


def _emit_mm1(nc, V, chunk, cix, svT, W1sb, psum1, l1v, b1sb):
    g0, g1 = chunk
    ng = g1 - g0
    for h in range(2):
        for q in range(16):
            k = q * 2 + h
            rhs = svT[h][:, g0 * 128:g1 * 128] \
                .rearrange("p (b a s) -> p a b s", b=ng, a=8)[:, :, :, q]
            nc.tensor.matmul(out=psum1[cix][:],
                             lhsT=W1sb[:, k * 128:(k + 1) * 128],
                             rhs=rhs, start=(h == 0 and q == 0),
                             stop=(h == 1 and q == 15))
    V.tensor_scalar(out=l1v[:, :, g0:g1], in0=psum1[cix][:],
                    scalar1=b1sb[:, 0:1], scalar2=0.0,
                    op0=OP.add, op1=OP.max)


def _emit_body(nc, tc, fm, prop, W1, b1, W2, b2, W3, b3, out, consts,
               cpool, wpool, ppool):
    ident_c, sel_c, cb_c, idx0_c_ap = consts
    V = nc.vector

    # ---------------- coords via selection matmul ----------------
    # psCB[p, (g, k)] = sum_k SEL[k, p] * prop_sm[k, (g, k4)] reconstructs
    # exact fp32 coords for partition block h = p//16 (hi+lo fp16 rows).
    phl = wpool.tile([16, 24], F16, name="phl")
    nc.sync.dma_start(phl[:], prop)
    sel = cpool.tile([16, 128], F16, name="sel")
    nc.scalar.dma_start(sel[:], sel_c)
    cb = cpool.tile([128, 8], F32, name="cb")
    nc.sync.dma_start(cb[:], cb_c)
    idx0s = cpool.tile([16, 1], I32, name="idx0")
    nc.sync.dma_start(idx0s[:], idx0_c_ap)
    idx0_c = idx0s[:]
    ident = cpool.tile([128, 128], F16, name="ident")
    psCB = ppool.tile([128, 24], F32, name="psCB")
    nc.tensor.matmul(out=psCB[:], lhsT=sel[:], rhs=phl[:], start=True,
                     stop=True)
    CB = cpool.tile([128, 24], F32, name="CB")
    V.tensor_copy(out=CB[:], in_=psCB[:])

    # scalar queue: remaining loads; W1 (the big one) last.
    nc.scalar.dma_start(ident[:], ident_c)
    W2sb = cpool.tile([128, HID2], F16, name="W2sb")
    nc.scalar.dma_start(W2sb[:], W2)
    W3sb = cpool.tile([HID2 + 1, NUM_CLASSES], F16, name="W3sb")
    nc.scalar.dma_start(W3sb[:], W3)
    b1sb = cpool.tile([128, 1], F32, name="b1sb")
    nc.scalar.dma_start(b1sb[:], b1.rearrange("(p o) -> p o", o=1))
    b2sb = cpool.tile([HID2, 1], F32, name="b2sb")
    nc.scalar.dma_start(b2sb[:], b2.rearrange("(p o) -> p o", o=1))
    W1sb = cpool.tile([128, 4096], F16, name="W1sb")
    nc.scalar.dma_start(W1sb[:], W1)

    # ---------------- index chain (critical path to the gathers) --------
    # Sample-major [128, (g, yx)] views.
    cgv = CB[:, :].rearrange("p (g k) -> p g k", g=NG)
    bofs = cb[:, 2:8]
    dyx = wpool.tile([128, 12], F32, name="dyx")
    syx = wpool.tile([128, 12], F32, name="syx")
    f0m = wpool.tile([128, 12], F32, name="f0m")
    f0 = wpool.tile([128, 12], F32, name="f0")
    pixb = wpool.tile([128, NG], F32, name="pixb")
    dv = dyx[:, :].rearrange("p (g yx) -> p g yx", yx=2)
    sv_ = syx[:, :].rearrange("p (g yx) -> p g yx", yx=2)
    f0v = f0[:, :].rearrange("p (g yx) -> p g yx", yx=2)
    cyx = cb[:, 0:2].unsqueeze(1).to_broadcast([128, NG, 2])

    V.tensor_tensor(out=dv[:], in0=cgv[:, :, 2:4], in1=cgv[:, :, 0:2],
                    op=OP.subtract)
    V.tensor_tensor(out=sv_[:], in0=dv[:], in1=cyx, op=OP.mult)
    V.tensor_tensor(out=sv_[:], in0=sv_[:], in1=cgv[:, :, 0:2], op=OP.add)
    # f0 = round(s - 0.5) via fp32 magic; consistent-pair bilinear stays exact
    V.tensor_scalar(out=f0m[:], in0=syx[:], scalar1=-0.5, scalar2=MAGIC,
                    op0=OP.add, op1=OP.add)
    V.tensor_scalar(out=f0[:], in0=f0m[:], scalar1=-MAGIC, scalar2=None,
                    op0=OP.add)
    # pixb = b*HP*W + y0*W + x0, clamped
    V.tensor_scalar(out=pixb[:], in0=f0v[:, :, 0], scalar1=float(W),
                    scalar2=None, op0=OP.mult)
    V.tensor_tensor(out=pixb[:], in0=pixb[:], in1=f0v[:, :, 1], op=OP.add)
    V.tensor_tensor(out=pixb[:], in0=pixb[:], in1=bofs, op=OP.add)
    V.tensor_scalar(out=pixb[:], in0=pixb[:], scalar1=0.0,
                    scalar2=float(MAX_PIX), op0=OP.max, op1=OP.min)
    idx = cpool.tile([128, NG], I32, name="gidx")
    V.tensor_copy(out=idx[:], in_=pixb[:])

    # warm up the SWDGE path so the first real gather doesn't pay ~1.2us
    # of cold-start: a tiny 16-descriptor indirect gather of row 0.
    warm = wpool.tile([128, 32], F16, name="warm")
    nc.gpsimd.indirect_dma_start(
        out=warm[0:16, :],
        out_offset=None,
        in_=fm.rearrange("b h w c -> (b h w) c"),
        in_offset=bass.IndirectOffsetOnAxis(ap=idx0_c, axis=0),
    )

    # ---------------- gathers: 6 indirect DMAs (128 descriptors) ---------
    # G[p, (g, x, ab, c)] fp16; each descriptor reads fmP rows idx, idx+1 =
    # pixels (x0, x0+1) x (row pair) x 256 ch. No gpsimd library needed.
    G = wpool.tile([128, NG * 1024], F16, name="gather")
    fmr = fm.rearrange("b h w c -> (b h w) c")            # [32512, 512]
    for j in range(NG):
        nc.gpsimd.indirect_dma_start(
            out=G[:, j * 1024:(j + 1) * 1024],
            out_offset=None,
            in_=fmr,
            in_offset=bass.IndirectOffsetOnAxis(ap=idx[:, j:j + 1], axis=0),
        )

    # ------- bilinear corner weights (off the gather critical path) -------
    lyx = wpool.tile([128, 12], F32, name="lyx")
    hyx = wpool.tile([128, 12], F32, name="hyx")
    V.tensor_tensor(out=lyx[:], in0=syx[:], in1=f0[:], op=OP.subtract)
    V.tensor_scalar(out=hyx[:], in0=lyx[:], scalar1=-1.0, scalar2=1.0,
                    op0=OP.mult, op1=OP.add)
    lv = lyx[:, :].rearrange("p (g yx) -> p g yx", yx=2)
    hv = hyx[:, :].rearrange("p (g yx) -> p g yx", yx=2)
    ly, lx = lv[:, :, 0], lv[:, :, 1]
    hy, hx = hv[:, :, 0], hv[:, :, 1]
    # wc[p, (g, x, ab)] fp16 (matches the fmP elem layout x-outer)
    wc = cpool.tile([128, 24], F16, name="wcat")
    wv = wc[:, :].rearrange("p (g x ab) -> p g x ab", x=2, ab=2)
    V.tensor_tensor(out=wv[:, :, 0, 0], in0=hy, in1=hx, op=OP.mult)
    V.tensor_tensor(out=wv[:, :, 0, 1], in0=ly, in1=hx, op=OP.mult)
    V.tensor_tensor(out=wv[:, :, 1, 0], in0=hy, in1=lx, op=OP.mult)
    V.tensor_tensor(out=wv[:, :, 1, 1], in0=ly, in1=lx, op=OP.mult)
    # wbig[cix]: wc chunk expanded over channels so the combine multiply
    # reads contiguous fp16 at full DVE rate (broadcast reads run ~2x slow).
    wbig = [wpool.tile([128, (g1 - g0) * 1024], F16, name=f"wbig{c}")
            for c, (g0, g1) in enumerate(CHUNKS)]
    for cix, (g0, g1) in enumerate(CHUNKS):
        ng = g1 - g0
        src = wc[:, g0 * 4:g1 * 4] \
            .rearrange("p (g x ab) -> p g x ab", x=2, ab=2).unsqueeze(4) \
            .to_broadcast([128, ng, 2, 2, C])
        dst = wbig[cix][:, :].rearrange("p (g x ab c) -> p g x ab c",
                                        g=ng, x=2, ab=2)
        nc.scalar.copy(out=dst, in_=src)

    # ---------------- bilinear combine + transpose, per 2-group chunk -----
    Gv = G[:, :].rearrange("p (g x ab c) -> p g x ab c", g=NG, x=2, ab=2)
    sv2 = wpool.tile([128, NG * 512], F16, name="sv2")
    sv = wpool.tile([128, NG * 256], F16, name="sv")
    s2v = sv2[:, :].rearrange("p (g x c) -> p g x c", g=NG, x=2)
    svv = sv[:, :].rearrange("p (g c) -> p g c", g=NG)
    svT = [wpool.tile([128, NG * 128], F16, name=f"svT{h}") for h in range(2)]
    # layer-1 psum per chunk: columns (a, b_chunk); l1 interleaves to (a, b)
    psum1 = [ppool.tile([128, 8 * (g1 - g0)], F32, name=f"psum1{c}")
             for c, (g0, g1) in enumerate(CHUNKS)]
    l1 = wpool.tile([128, NRS], F16, name="l1")
    l1v = l1[:, :].rearrange("p (a b) -> p a b", a=8)
    # combine + transpose per GROUP as each gather lands; matmuls per pair
    for g in range(NG):
        cix, g0 = g // 2, (g // 2) * 2
        gs = slice(g, g + 1)
        wco = (g - g0) * 1024
        V.tensor_tensor(out=Gv[:, gs], in0=Gv[:, gs],
                        in1=wbig[cix][:, wco:wco + 1024].rearrange(
                            "p (g x ab c) -> p g x ab c", g=1, x=2, ab=2),
                        op=OP.mult)
        V.tensor_tensor(out=s2v[:, gs], in0=Gv[:, gs, :, 0], in1=Gv[:, gs, :, 1],
                        op=OP.add)
        V.tensor_tensor(out=svv[:, gs], in0=s2v[:, gs, 0], in1=s2v[:, gs, 1],
                        op=OP.add)
        for h in range(2):
            pt = ppool.tile([128, 128], F16, tag="pt", bufs=3, name="pt")
            nc.tensor.transpose(
                out=pt[:],
                in_=sv[:, g * 256 + h * 128: g * 256 + (h + 1) * 128],
                identity=ident[:])
            nc.scalar.copy(out=svT[h][:, g * 128:(g + 1) * 128], in_=pt[:])
        if g % 2 == 1:
            _emit_mm1(nc, V, CHUNKS[cix], cix, svT, W1sb, psum1, l1v, b1sb)

    # ---------------- MLP layers 2, 3 ----------------
    psum23 = ppool.tile([HID2 + 1, NRS + NUM_CLASSES], F32, name="psum23")
    psum2 = psum23[0:HID2, 0:NRS]
    psum3 = psum23[0:NRS, NRS:NRS + NUM_CLASSES]
    nc.tensor.matmul(out=psum2, lhsT=W2sb[:, :], rhs=l1[:], start=True,
                     stop=True)
    l2 = wpool.tile([HID2 + 1, NRS], F16, name="l2")
    # ones row 64 folds b3 into the W3 matmul (W3sb row 64 = b3)
    V.tensor_scalar(out=l2[64:65, :], in0=ident[64:65, 0:NRS], scalar1=0.0,
                    scalar2=1.0, op0=OP.mult, op1=OP.add)
    V.tensor_scalar(out=l2[0:HID2, :], in0=psum2, scalar1=b2sb[:, 0:1],
                    scalar2=0.0, op0=OP.add, op1=OP.max)
    nc.tensor.matmul(out=psum3, lhsT=l2[:], rhs=W3sb[:], start=True,
                     stop=True)

    # ---------------- softmax (rows 0..43 only, fp32) ----------------
    # logits are O(10), so fp32 exp needs no max-subtraction; b3 is already
    # folded into psum3 via the l2 ones row.
    ex = wpool.tile([NROI, NUM_CLASSES], F32, name="ex")
    nc.scalar.activation(out=ex[:], in_=psum3[0:NROI, :], func=AF.Exp,
                         bias=0.0, scale=1.0)
    ssum = wpool.tile([NROI, 1], F32, name="ssum")
    V.tensor_reduce(out=ssum[:], in_=ex[:], axis=AX_X, op=OP.add)
    rinv = wpool.tile([NROI, 1], F32, name="rinv")
    V.reciprocal(rinv[:], ssum[:])
    probs = wpool.tile([NROI, NUM_CLASSES], F32, name="probs")
    V.tensor_scalar(out=probs[:], in0=ex[:], scalar1=rinv[:, 0:1],
                    scalar2=None, op0=OP.mult)

    nc.sync.dma_start(out.rearrange("b p c -> (b p) c"), probs[:])


def build_module():
    nc = bacc.Bacc(get_trn_type() or "TRN2", target_bir_lowering=False, debug=False)
    fm = nc.dram_tensor("feature_map", [B_LOC, HP, W, 2 * C], F16, kind="ExternalInput")
    prop = nc.dram_tensor("proposals", [16, 24], F16, kind="ExternalInput")
    W1 = nc.dram_tensor("W1", [128, 4096], F16, kind="ExternalInput")
    b1 = nc.dram_tensor("b1", [HID1], F32, kind="ExternalInput")
    W2 = nc.dram_tensor("W2", [HID1, HID2], F16, kind="ExternalInput")
    b2 = nc.dram_tensor("b2", [HID2], F32, kind="ExternalInput")
    W3 = nc.dram_tensor("W3", [HID2 + 1, NUM_CLASSES], F16, kind="ExternalInput")
    b3 = nc.dram_tensor("b3", [NUM_CLASSES], F32, kind="ExternalInput")
    out = nc.dram_tensor("out", [B_LOC, P, NUM_CLASSES], F32, kind="ExternalOutput")

    ident_np, sel_np, cb_np, idx0_np = _static_consts()
    ident_c = nc.inline_tensor(ident_np, name="c_ident")
    sel_c = nc.inline_tensor(sel_np, name="c_sel")
    cb_c = nc.inline_tensor(cb_np, name="c_cb")
    idx0_c = nc.inline_tensor(idx0_np, name="c_idx0")

    with tile.TileContext(nc) as tc:
        emit_kernel(nc, tc, fm[:], prop[:], W1[:], b1[:], W2[:], b2[:], W3[:],
                    b3[:], out[:], (ident_c[:], sel_c[:], cb_c[:], idx0_c[:]))
    nc.compile()
    return nc


_NC_CACHE = None


def _get_module():
    global _NC_CACHE
    if _NC_CACHE is None:
        _NC_CACHE = build_module()
    return _NC_CACHE


def _shard_inputs(inputs):
    fm16 = np.asarray(inputs["feature_map"], dtype=np.float32).astype(np.float16)
    # paired rows: fmP[b, y, x] = fm[b, y] ++ fm[b, y+1] per pixel
    fmP = np.concatenate([fm16[:, :-1], fm16[:, 1:]], axis=3)
    fmP = np.ascontiguousarray(fmP)
    props = np.asarray(inputs["proposals"], dtype=np.float32)
    # W1 rows k*128+p -> [p, k*128+j] fp16 so lhsT chunks are contiguous.
    W1h = np.ascontiguousarray(
        np.asarray(inputs["W1"], dtype=np.float32).reshape(32, 128, HID1)
        .transpose(1, 0, 2).reshape(128, 4096).astype(np.float16))
    W2h = np.ascontiguousarray(np.asarray(inputs["W2"], dtype=np.float32).astype(np.float16))
    W3h = np.ascontiguousarray(np.concatenate(
        [np.asarray(inputs["W3"], dtype=np.float32),
         np.asarray(inputs["b3"], dtype=np.float32)[None, :]], axis=0)
        .astype(np.float16))
    b1h = np.ascontiguousarray(np.asarray(inputs["b1"], dtype=np.float32))
    b2h = np.ascontiguousarray(np.asarray(inputs["b2"], dtype=np.float32))
    b3h = np.ascontiguousarray(np.asarray(inputs["b3"], dtype=np.float32))
    in_maps = []
    for c in range(N_CORES):
        sl = slice(B_LOC * c, B_LOC * (c + 1))
        # prop_sm[2h]   = fp16 hi of coords for roi slots (h, g=0..5)
        # prop_sm[2h+1] = fp16 lo; h=7 slots hold rois [42, 43, 38..41].
        pf = props[sl].reshape(NROI, 4)
        rows = np.zeros((8, 6, 4), np.float32)
        for h in range(7):
            rows[h] = pf[h * 6:(h + 1) * 6]
        rows[7, 0:2] = pf[42:44]
        rows[7, 2:6] = pf[38:42]
        rows = rows.reshape(8, 24)
        hi = rows.astype(np.float16)
        lo = (rows - hi.astype(np.float32)).astype(np.float16)
        phl = np.zeros((16, 24), np.float16)
        phl[0::2] = hi
        phl[1::2] = lo
        in_maps.append({
            "feature_map": fmP[sl],
            "proposals": np.ascontiguousarray(phl),
            "W1": W1h, "b1": b1h,
            "W2": W2h, "b2": b2h,
            "W3": W3h, "b3": b3h,
        })
    return in_maps


def run(inputs, trace=False):
    """Run on all 8 cores; returns (output [16,22,10], BassKernelResults)."""
    nc = _get_module()
    res = run_bass_kernel_spmd(nc, _shard_inputs(inputs), core_ids=list(range(N_CORES)),
                               trace=trace)
    out = np.concatenate([r["out"] for r in res.results], axis=0)
    return out, res


def kernel(**inputs) -> np.ndarray:
    out, _ = run(inputs, trace=False)
    return out
